# revision 8
# baseline (speedup 1.0000x reference)
"""Trainium2 Bass kernel for nn_Kernel_Layer_55654186221960.

Computes y = einsum('bmi,mio->bmo', x, weights) with
x (4096, 16, 512) f32 and weights (16, 512, 512) f32.

Distribution: the 16 independent m-groups are sharded 2-per-core across
8 NeuronCores (better than batch-parallel: each core only needs its own
2.1 MB weight slice instead of the full 16.8 MB, cutting HBM traffic).

Per-core kernel (SPMD, x shard (4096, 2, 512), w shard (2, 512, 512)):
  for each 128-row batch tile:
    - DMA x tile [128b, 2m, 512i] to SBUF (4 KB/partition contiguous)
    - PE-transpose the 4 [128,128] k-slices (fp32 can't DMA-transpose)
    - 4 accumulating matmuls  out[128b, 512o] += xT[k].T @ w[k]
      (fp32r: reduced-precision fp32 PE mode, 4x faster than fp32)
    - PSUM -> SBUF copy on ACT, DMA out (4 KB/partition contiguous)
"""

import sys

for _p in ("/opt/trn_rl_repo",):
    if _p not in sys.path:
        sys.path.insert(0, _p)

import numpy as np

import concourse.bass as bass
import concourse.mybir as mybir
import concourse.tile as tile
from concourse import bacc
from concourse.bass_utils import run_bass_kernel_spmd

B, M, D = 4096, 16, 512
NCORES = 8
MG = M // NCORES          # m-groups per core = 2
P = 128
KT = D // P               # 4 k-tiles along d_in
BT = B // P               # 32 batch tiles
F32 = mybir.dt.float32

_built = {}


def _build(mm_dtype_name="float32r"):
    mmdt = getattr(mybir.dt, mm_dtype_name)
    nc = bacc.Bacc("TRN2", target_bir_lowering=False, debug=False)
    # x/w/ident declared at the matmul dtype (float32r is fp32 bits with
    # reduced-precision PE rounding; walrus requires every producer
    # feeding an fp32r matmul to be fp32r-typed).
    x_d = nc.dram_tensor("x", [B, MG, D], mmdt, kind="ExternalInput").ap()
    w_d = nc.dram_tensor("w", [MG, D, D], mmdt, kind="ExternalInput").ap()
    i_d = nc.dram_tensor("ident", [P, P], mmdt, kind="ExternalInput").ap()
    y_d = nc.dram_tensor("y", [B, MG, D], F32, kind="ExternalOutput").ap()

    with tile.TileContext(nc) as tc:
        with (
            tc.tile_pool(name="const", bufs=1) as cpool,
            tc.tile_pool(name="wpool", bufs=1) as wpool,
            tc.tile_pool(name="xin", bufs=4) as xpool,
            tc.tile_pool(name="xt", bufs=4) as xtpool,
            tc.tile_pool(name="yout", bufs=4) as ypool,
            tc.tile_pool(name="tps", bufs=2, space=bass.MemorySpace.PSUM) as tpsum,
            tc.tile_pool(name="ops", bufs=4, space=bass.MemorySpace.PSUM) as opsum,
        ):
            # ident/weight loads ride the ACT HWDGE ring (idle early) so the
            # first x tiles start flowing on the SP ring immediately.
            ident = cpool.tile([P, P], mmdt)
            nc.scalar.dma_start(ident[:], i_d[:])

            # Weights resident in SBUF for the whole kernel (16 KB/partition).
            w_sb = wpool.tile([P, MG, KT, D], mmdt)
            for m in range(MG):
                for k in range(KT):
                    nc.scalar.dma_start(w_sb[:, m, k, :], w_d[m, k * P:(k + 1) * P, :])

            for bt in range(BT):
                x_nat = xpool.tile([P, MG, D], mmdt)
                nc.sync.dma_start(x_nat[:], x_d[bt * P:(bt + 1) * P, :, :])
                for m in range(MG):
                    # x tile [128b, 512i] -> xT [128i, 4k, 128b] via PE transpose
                    pst = tpsum.tile([P, KT, P], mmdt)
                    for k in range(KT):
                        nc.tensor.transpose(
                            pst[:, k, :], x_nat[:, m, k * P:(k + 1) * P], ident[:]
                        )
                    xt = xtpool.tile([P, KT, P], mmdt)
                    nc.vector.tensor_copy(xt[:], pst[:])

                    out_ps = opsum.tile([P, D], F32)
                    for k in range(KT):
                        nc.tensor.matmul(
                            out_ps[:],
                            xt[:, k, :],
                            w_sb[:, m, k, :],
                            start=(k == 0),
                            stop=(k == KT - 1),
                        )
                    y_sb = ypool.tile([P, D], F32, tag="ysb")
                    nc.scalar.copy(y_sb[:], out_ps[:])
                    # per-m output DMAs on the ACT HWDGE ring: finer-grained
                    # drain keeps DMA fed during the PE-paced tail, and the
                    # two streams don't share the SP ring's issue FIFO.
                    nc.scalar.dma_start(y_d[bt * P:(bt + 1) * P, m, :], y_sb[:])

    nc.compile()
    return nc


def _get(mm_dtype_name="float32r"):
    if mm_dtype_name not in _built:
        _built[mm_dtype_name] = _build(mm_dtype_name)
    return _built[mm_dtype_name]


def _run(x, weights, mm_dtype_name="float32r", **spmd_kwargs):
    x = np.ascontiguousarray(np.asarray(x, dtype=np.float32))
    w = np.ascontiguousarray(np.asarray(weights, dtype=np.float32))
    assert x.shape == (B, M, D) and w.shape == (M, D, D)
    nc = _get(mm_dtype_name)
    ident = np.eye(P, dtype=np.float32)
    in_maps = []
    for c in range(NCORES):
        ms = slice(c * MG, (c + 1) * MG)
        in_maps.append(
            {
                "x": np.ascontiguousarray(x[:, ms, :]),
                "w": np.ascontiguousarray(w[ms]),
                "ident": ident,
            }
        )
    res = run_bass_kernel_spmd(nc, in_maps, list(range(NCORES)), **spmd_kwargs)
    y = np.empty((B, M, D), np.float32)
    for c in range(NCORES):
        y[:, c * MG:(c + 1) * MG, :] = res.results[c]["y"]
    return y, res


def kernel(x, weights):
    y, _ = _run(x, weights)
    return y
